# revision 7
# baseline (speedup 1.0000x reference)
"""DETR self-attention (B=4, T=2048, E=1024, H=16) on 8 trn2 NeuronCores.

Sharding: core c handles batch c//2 and query-half c%2 (1024 query rows),
computing K/V for the full 2048-token sequence of its batch (duplicated
across the pair of cores — cheaper than an intra-pair collective).

v2 design (serial phases, 1024-wide moving dims, minimal instruction count):
  phase A: stream hidT/oqT columns; hq = hid+oq (bf16); v-proj (bf16,
           one 1024-wide chunk per s-tile); kT/qT = W.T-stationary @ hqT
           with all weights resident; kq ch0 units interleave into the
           back half of the column stream, ch1 units after.
           bv is folded into bo on the host (attn rows sum to 1 =>
           bv contributes exactly bv @ Wo.T to the output).
  phase B: per head: 16 scores matmuls (d=64 contraction, 1024-wide)
           -> exp on ACT (scale=1/8 folded) -> lagged attn@v into a
           [65, 1024] PSUM accumulator (row 64 = softmax Z via ones col).
           1/Z via DVE reciprocal, broadcast via k=1 PE matmul, final
           psv * (1/Z) -> outT bf16.
  phase C: out_proj: y[t, e] = outT.T @ WoT + bo2, DMA out per t-tile.
exp(scores) never overflows: scores*0.125 ~ N(0, 0.82), |max| < 6.
"""
import os
import sys

if "/opt/trn_rl_repo" not in sys.path:
    sys.path.insert(0, "/opt/trn_rl_repo")

from contextlib import ExitStack, nullcontext

import numpy as np

import concourse.bass as bass
import concourse.tile as tile
from concourse import bacc, mybir
from concourse.bass_utils import run_bass_kernel_spmd

F32 = mybir.dt.float32
F32R = mybir.dt.float32r
BF16 = mybir.dt.bfloat16
EXP = mybir.ActivationFunctionType.Exp

B, T, E, H, D = 4, 2048, 1024, 16, 64
TL = T // 2          # local query rows per core
N_CORES = 8
KT = 128             # contraction tile
MT = E // KT         # 8 e-blocks
ST = T // KT         # 16 s-tiles
SCALE = 1.0 / 8.0    # D ** -0.5
LAG = int(os.environ.get('K_LAG', 2))


def build_program(repeat=1, only="full"):
    nc = bacc.Bacc("TRN2", target_bir_lowering=False, debug=False)

    import concourse.mybir as _mb

    hidT = nc.dram_tensor("hidT", [128, ST, MT, KT], BF16,
                          kind="ExternalInput").ap()
    oqT = nc.dram_tensor("oqT", [128, ST, MT, KT], BF16,
                         kind="ExternalInput").ap()
    wqT = nc.dram_tensor("wqT", [128, MT, MT, KT], BF16,
                         kind="ExternalInput").ap()
    wkT = nc.dram_tensor("wkT", [128, MT, MT, KT], BF16,
                         kind="ExternalInput").ap()
    wvT = nc.dram_tensor("wvT", [E, E], BF16, kind="ExternalInput").ap()
    woT = nc.dram_tensor("woT", [E, E], BF16, kind="ExternalInput").ap()
    bq = nc.dram_tensor("bq", [128, MT], F32, kind="ExternalInput").ap()
    bk = nc.dram_tensor("bk", [128, MT], F32, kind="ExternalInput").ap()
    bo2 = nc.dram_tensor("bo2", [1, E], BF16, kind="ExternalInput").ap()
    onesf = nc.dram_tensor("onesf", [1, KT], F32R, kind="ExternalInput").ap()
    onesb = nc.dram_tensor("onesb", [1, KT], BF16, kind="ExternalInput").ap()
    onebf = nc.dram_tensor("onebf", [128, ST * H], BF16, kind="ExternalInput").ap()
    y = nc.dram_tensor("y", [TL, E], F32, kind="ExternalOutput").ap()

    with tile.TileContext(nc, pool_alloc_mode="queue") as tc, \
            (tc.For_i(0, repeat, 1) if repeat > 1 else nullcontext()), \
            ExitStack() as top:
        misc = top.enter_context(tc.tile_pool(name="misc", bufs=1))
        kq_pool = top.enter_context(tc.tile_pool(name="kq", bufs=1))
        v_pool = top.enter_context(tc.tile_pool(name="vp", bufs=1))

        # --- constants / biases ---
        ones65 = misc.tile([65, 64], F32R, tag="ones65")
        nc.sync.dma_start(ones65[64:65, :], onesf[0:1, 0:64])
        ones_b = misc.tile([1, KT], BF16, tag="onesb")
        nc.sync.dma_start(ones_b[:], onesb[:])
        bq_t = misc.tile([128, MT], F32, tag="bq")
        nc.sync.dma_start(bq_t[:], bq[:])
        bk_t = misc.tile([128, MT], F32, tag="bk")
        nc.sync.dma_start(bk_t[:], bk[:])
        bo2_t = misc.tile([1, E], BF16, tag="bo2")
        nc.sync.dma_start(bo2_t[:], bo2[:])

        # --- resident stores ---
        kT_sb = kq_pool.tile([128, MT, T], BF16, tag="kT")
        qT_sb = kq_pool.tile([128, MT, TL], BF16, tag="qT")
        v_sb = v_pool.tile([128, ST, H, 65], BF16, tag="v")
        nc.sync.dma_start(v_sb[:, :, :, 64:65], onebf[:])

        # =========== phase A: v-proj + hq + kq ==============================
        if only != "c":
            hq_ctx = tc.tile_pool(name="hqp", bufs=1)
            hq_pool = hq_ctx.__enter__()
            hq_sb = hq_pool.tile([128, MT, T], BF16, tag="hq")
            w_ctx = tc.tile_pool(name="wts", bufs=1)
            w_pool = w_ctx.__enter__()
            wv_sb = w_pool.tile([128, MT, E], BF16, tag="wv")
            wk_sb = w_pool.tile([128, MT, MT, KT], BF16, tag="wk")
            wq_sb = w_pool.tile([128, MT, MT, KT], BF16, tag="wq")
            col_ctx = tc.tile_pool(name="col", bufs=2)
            col_pool = col_ctx.__enter__()
            vps_ctx = tc.tile_pool(name="vps", bufs=2, space="PSUM")
            vps = vps_ctx.__enter__()
            kqp_ctx = tc.tile_pool(name="kqp", bufs=2, space="PSUM")
            kqp = kqp_ctx.__enter__()

            for k in range(MT):
                nc.sync.dma_start(wv_sb[:, k, :], wvT[k * KT:(k + 1) * KT, :])
            nc.sync.dma_start(wk_sb[:], wkT[:])
            nc.sync.dma_start(wq_sb[:], wqT[:])

            def col_load(i):
                hc = col_pool.tile([128, MT, KT], BF16, tag="hc", name="hc")
                nc.sync.dma_start(hc[:], hidT[:, i])
                oc = col_pool.tile([128, MT, KT], BF16, tag="oc", name="oc")
                nc.sync.dma_start(oc[:], oqT[:, i])
                return hc, oc

            def kq_unit(m, which):
                # which: 0 = kT ch0, 1 = kT ch1, 2 = qT
                w = wq_sb if which == 2 else wk_sb
                t0 = TL if which == 1 else 0
                cols = slice(t0, t0 + TL)
                ps = kqp.tile([128, TL], F32, tag="kqp", name="ps")
                for k in range(MT):
                    for c in range(2):  # ISA caps matmul moving dim at 512
                        nc.tensor.matmul(
                            ps[:, 512 * c:512 * (c + 1)], w[:, m, k, :],
                            hq_sb[:, k, t0 + 512 * c:t0 + 512 * (c + 1)],
                            start=(k == 0), stop=(k == MT - 1))
                if which == 2:
                    nc.vector.tensor_scalar(
                        qT_sb[:, m, :], ps[:], bq_t[:, m:m + 1],
                        None, _mb.AluOpType.add)
                else:
                    nc.vector.tensor_scalar(
                        kT_sb[:, m, cols], ps[:], bk_t[:, m:m + 1],
                        None, _mb.AluOpType.add)

            # ch0 units (need hq cols 0..7): 16 of them -> 2 per i for i>=8
            ch0_units = []
            for m in range(MT):
                ch0_units.append((m, 0))
                ch0_units.append((m, 2))

            nxt = col_load(0)
            for i in range(ST):
                hc, oc = nxt
                if i + 1 < ST:
                    nxt = col_load(i + 1)
                nc.vector.tensor_add(hq_sb[:, :, i * KT:(i + 1) * KT],
                                     hc[:], oc[:])
                ps = vps.tile([128, E], F32, tag="vps", name="ps")
                for k in range(MT):
                    for c in range(2):
                        nc.tensor.matmul(
                            ps[:, 512 * c:512 * (c + 1)], hc[:, k, :],
                            wv_sb[:, k, 512 * c:512 * (c + 1)],
                            start=(k == 0), stop=(k == MT - 1))
                nc.vector.tensor_copy(
                    v_sb[:, i, :, 0:64],
                    ps[:].rearrange("p (h d) -> p h d", d=64))
                if i >= 8:
                    kq_unit(*ch0_units[2 * (i - 8)])
                    kq_unit(*ch0_units[2 * (i - 8) + 1])
            for m in range(MT):
                kq_unit(m, 1)

            kqp_ctx.__exit__(None, None, None)
            vps_ctx.__exit__(None, None, None)
            col_ctx.__exit__(None, None, None)
            w_ctx.__exit__(None, None, None)
            hq_ctx.__exit__(None, None, None)

        # =========== phase B: attention =====================================
        outT_ctx = tc.tile_pool(name="outT", bufs=1, side="right")
        outT_pool = outT_ctx.__enter__()
        outT_sb = outT_pool.tile([128, MT, TL], BF16, tag="outT")

        if only == "c":
            nc.vector.memset(kT_sb[:, 0, 0:16], 0.5)
            nc.vector.memset(qT_sb[:, 0, 0:16], 0.5)
            nc.vector.memset(v_sb[:, 0, 0, 0:16], 0.5)

        if only != "ab":
            exp_ctx = tc.tile_pool(name="expp", bufs=6)
            exp_pool = exp_ctx.__enter__()
            z_ctx = tc.tile_pool(name="zp", bufs=2)
            z_pool = z_ctx.__enter__()
            sc_ctx = tc.tile_pool(name="sc", bufs=2, space="PSUM")
            sc = sc_ctx.__enter__()
            pv_ctx = tc.tile_pool(name="pv", bufs=2, space="PSUM")
            pv = pv_ctx.__enter__()

            for p in range(MT):
                for e in range(2):
                    h = 2 * p + e
                    lo, hi = 64 * e, 64 * e + 64
                    psv = pv.tile([65, TL], F32, tag="pv", name="psv")
                    exs = [None] * ST
                    def att_mm(j, start, stop):
                        for c in range(2):
                            nc.tensor.matmul(
                                psv[:, 512 * c:512 * (c + 1)],
                                v_sb[:, j, h, :],
                                exs[j][:, 512 * c:512 * (c + 1)],
                                start=start, stop=stop)

                    for i in range(ST):
                        psc = sc.tile([128, TL], F32, tag="sc", name="psc")
                        for c in range(2):
                            nc.tensor.matmul(
                                psc[:, 512 * c:512 * (c + 1)],
                                kT_sb[lo:hi, p, i * KT:(i + 1) * KT],
                                qT_sb[lo:hi, p, 512 * c:512 * (c + 1)],
                                start=True, stop=True)
                        ex = exp_pool.tile([128, TL], BF16, tag="exp",
                                           name="ex")
                        nc.scalar.activation(ex[:], psc[:], EXP, scale=SCALE)
                        exs[i] = ex
                        j = i - LAG
                        if j >= 0:
                            att_mm(j, j == 0, j == ST - 1)
                            exs[j] = None
                    for j in range(ST - LAG, ST):
                        att_mm(j, j == 0, j == ST - 1)
                    rz = z_pool.tile([65, TL], F32R, tag="rz", name="rz")
                    with nc.allow_low_precision(reason="softmax recip"):
                        nc.vector.reciprocal(rz[64:65, :], psv[64:65, :])
                    pzb = sc.tile([128, TL], F32, tag="sc", name="pzb")
                    for c in range(2):
                        nc.tensor.matmul(pzb[0:64, 512 * c:512 * (c + 1)],
                                         ones65[64:65, :],
                                         rz[64:65, 512 * c:512 * (c + 1)],
                                         start=True, stop=True)
                    zbs = z_pool.tile([64, TL], F32, tag="zbs", name="zbs")
                    nc.vector.tensor_copy(zbs[:], pzb[0:64, :])
                    nc.vector.tensor_mul(
                        outT_sb[lo:hi, p, :], psv[0:64, :], zbs[:])

            pv_ctx.__exit__(None, None, None)
            sc_ctx.__exit__(None, None, None)
            z_ctx.__exit__(None, None, None)
            exp_ctx.__exit__(None, None, None)

        # =========== phase C: out_proj ======================================
        if only == "ab":
            nc.vector.memset(outT_sb[:, 0, 0:16], 0.5)
        with tc.tile_pool(name="wo", bufs=1) as wo_pool, \
             tc.tile_pool(name="yo", bufs=3) as y_pool, \
             tc.tile_pool(name="yp", bufs=3, space="PSUM") as yp:
            wo_sb = wo_pool.tile([128, MT, E], BF16, tag="wo")
            for k in range(MT):
                nc.sync.dma_start(wo_sb[:, k, :], woT[k * KT:(k + 1) * KT, :])
            for tt in range(TL // KT):
                ps = yp.tile([128, E], F32, tag="yp", name="ps")
                for k in range(MT):
                    for c in range(2):
                        nc.tensor.matmul(
                            ps[:, 512 * c:512 * (c + 1)],
                            outT_sb[:, k, tt * KT:(tt + 1) * KT],
                            wo_sb[:, k, 512 * c:512 * (c + 1)],
                            start=(k == 0), stop=False)
                for c in range(2):
                    nc.tensor.matmul(
                        ps[:, 512 * c:512 * (c + 1)], ones_b[0:1, :],
                        bo2_t[0:1, 512 * c:512 * (c + 1)],
                        start=False, stop=True)
                yt = y_pool.tile([128, E], F32, tag="yt", name="yt")
                nc.vector.tensor_copy(yt[:], ps[:])
                nc.sync.dma_start(y[tt * KT:(tt + 1) * KT, :], yt[:])
        outT_ctx.__exit__(None, None, None)

    nc.compile()
    return nc


_NC_CACHE = None


def _get_program():
    global _NC_CACHE
    if _NC_CACHE is None:
        _NC_CACHE = build_program()
    return _NC_CACHE


def _bf16_np():
    import ml_dtypes
    return ml_dtypes.bfloat16


def make_in_maps(hidden_states, object_queries, Wq, bq, Wk, bk, Wv, bv, Wo, bo):
    """Host-side sharding/layout prep -> per-core input dicts."""
    bf = _bf16_np()
    bo2 = bo + bv @ Wo.T
    shared = {
        "wqT": np.ascontiguousarray(
            Wq.T.reshape(MT, 128, MT, KT).transpose(1, 2, 0, 3)).astype(bf),
        "wkT": np.ascontiguousarray(
            Wk.T.reshape(MT, 128, MT, KT).transpose(1, 2, 0, 3)).astype(bf),
        "wvT": np.ascontiguousarray(Wv.T).astype(bf),
        "woT": np.ascontiguousarray(Wo.T).astype(bf),
        "bq": np.ascontiguousarray(bq.reshape(MT, 128).T),
        "bk": np.ascontiguousarray(bk.reshape(MT, 128).T),
        "bo2": bo2[None, :].astype(bf),
        "onesf": np.ones((1, KT), np.float32),
        "onesb": np.ones((1, KT), bf),
        "onebf": np.ones((128, ST * H), bf),
    }
    in_maps = []
    for c in range(N_CORES):
        b, half = c // 2, c % 2
        toff = half * TL
        # rotate T so this core's query rows come first (attention over s is
        # permutation-invariant as long as k/v share the ordering)
        hid = np.concatenate([hidden_states[b, toff:], hidden_states[b, :toff]], 0)
        oq = np.concatenate([object_queries[b, toff:], object_queries[b, :toff]], 0)
        m = dict(shared)
        m["hidT"] = np.ascontiguousarray(
            hid.T.reshape(MT, 128, ST, KT).transpose(1, 2, 0, 3)).astype(bf)
        m["oqT"] = np.ascontiguousarray(
            oq.T.reshape(MT, 128, ST, KT).transpose(1, 2, 0, 3)).astype(bf)
        in_maps.append(m)
    return in_maps


def kernel(**inputs):
    nc = _get_program()
    in_maps = make_in_maps(**{k: np.asarray(v) for k, v in inputs.items()})
    res = run_bass_kernel_spmd(nc, in_maps, core_ids=list(range(N_CORES)))
    out = np.empty((B, T, E), np.float32)
    for c in range(N_CORES):
        b, half = c // 2, c % 2
        out[b, half * TL:(half + 1) * TL] = res.results[c]["y"]
    return out


# revision 11
# speedup vs baseline: 108.6332x; 108.6332x over previous
"""DETR self-attention (B=4, T=2048, E=1024, H=16) on 8 trn2 NeuronCores.

Sharding: core c handles batch c//2 and query-half c%2 (1024 query rows),
computing K/V for the full 2048-token sequence of its batch (duplicated
across the pair of cores — cheaper than an intra-pair collective).

v2 design (serial phases, 1024-wide moving dims, minimal instruction count):
  phase A: stream hidT/oqT columns; hq = hid+oq (bf16); v-proj (bf16,
           one 1024-wide chunk per s-tile); kT/qT = W.T-stationary @ hqT
           with all weights resident; kq ch0 units interleave into the
           back half of the column stream, ch1 units after.
           bv is folded into bo on the host (attn rows sum to 1 =>
           bv contributes exactly bv @ Wo.T to the output).
  phase B: per head: 16 scores matmuls (d=64 contraction, 1024-wide)
           -> exp on ACT (scale=1/8 folded) -> lagged attn@v into a
           [65, 1024] PSUM accumulator (row 64 = softmax Z via ones col).
           1/Z via DVE reciprocal, broadcast via k=1 PE matmul, final
           psv * (1/Z) -> outT bf16.
  phase C: out_proj: y[t, e] = outT.T @ WoT + bo2, DMA out per t-tile.
exp(scores) never overflows: scores*0.125 ~ N(0, 0.82), |max| < 6.
"""
import os
import sys

if "/opt/trn_rl_repo" not in sys.path:
    sys.path.insert(0, "/opt/trn_rl_repo")

from contextlib import ExitStack, nullcontext

import numpy as np

import concourse.bass as bass
import concourse.tile as tile
from concourse import bacc, mybir
from concourse.bass_utils import run_bass_kernel_spmd

F32 = mybir.dt.float32
F32R = mybir.dt.float32r
BF16 = mybir.dt.bfloat16
EXP = mybir.ActivationFunctionType.Exp

B, T, E, H, D = 4, 2048, 1024, 16, 64
TL = T // 2          # local query rows per core
N_CORES = 8
KT = 128             # contraction tile
MT = E // KT         # 8 e-blocks
ST = T // KT         # 16 s-tiles
SCALE = 1.0 / 8.0    # D ** -0.5
LAG = int(os.environ.get('K_LAG', 2))


def build_program(repeat=1, only="full"):
    nc = bacc.Bacc("TRN2", target_bir_lowering=False, debug=False)

    import concourse.mybir as _mb

    hidT = nc.dram_tensor("hidT", [128, ST, MT, KT], BF16,
                          kind="ExternalInput").ap()
    oqT = nc.dram_tensor("oqT", [128, ST, MT, KT], BF16,
                         kind="ExternalInput").ap()
    wqT = nc.dram_tensor("wqT", [128, MT, MT, KT], BF16,
                         kind="ExternalInput").ap()
    wkT = nc.dram_tensor("wkT", [128, MT, MT, KT], BF16,
                         kind="ExternalInput").ap()
    wvT = nc.dram_tensor("wvT", [E, E], BF16, kind="ExternalInput").ap()
    woT = nc.dram_tensor("woT", [E, E], BF16, kind="ExternalInput").ap()
    bq = nc.dram_tensor("bq", [128, MT], F32, kind="ExternalInput").ap()
    bk = nc.dram_tensor("bk", [128, MT], F32, kind="ExternalInput").ap()
    bo2 = nc.dram_tensor("bo2", [1, E], BF16, kind="ExternalInput").ap()
    onesf = nc.dram_tensor("onesf", [1, KT], F32R, kind="ExternalInput").ap()
    onesb = nc.dram_tensor("onesb", [1, KT], BF16, kind="ExternalInput").ap()
    onebf = nc.dram_tensor("onebf", [128, ST * H], BF16, kind="ExternalInput").ap()
    y = nc.dram_tensor("y", [TL, E], F32, kind="ExternalOutput").ap()

    with tile.TileContext(nc, pool_alloc_mode="queue") as tc, \
            (tc.For_i(0, repeat, 1) if repeat > 1 else nullcontext()), \
            ExitStack() as top:
        misc = top.enter_context(tc.tile_pool(name="misc", bufs=1))
        kq_pool = top.enter_context(tc.tile_pool(name="kq", bufs=1))
        v_pool = top.enter_context(tc.tile_pool(name="vp", bufs=1))

        # --- constants / biases ---
        ones65 = misc.tile([65, 64], F32R, tag="ones65")
        nc.sync.dma_start(ones65[64:65, :], onesf[0:1, 0:64])
        ones_b = misc.tile([1, KT], BF16, tag="onesb")
        nc.sync.dma_start(ones_b[:], onesb[:])
        bq_t = misc.tile([128, MT], F32, tag="bq")
        nc.sync.dma_start(bq_t[:], bq[:])
        bk_t = misc.tile([128, MT], F32, tag="bk")
        nc.sync.dma_start(bk_t[:], bk[:])
        bo2_t = misc.tile([1, E], BF16, tag="bo2")
        nc.sync.dma_start(bo2_t[:], bo2[:])

        # --- resident stores ---
        kT_sb = kq_pool.tile([128, MT, T], BF16, tag="kT")
        qT_sb = kq_pool.tile([128, MT, TL], BF16, tag="qT")
        v_sb = v_pool.tile([128, ST, H, 65], BF16, tag="v")
        nc.sync.dma_start(v_sb[:, :, :, 64:65], onebf[:])

        # =========== phase A: v-proj + hq + kq ==============================
        N_PRE = 3  # kq blocks computed in phase A; the rest fill attention
        hq_ctx = w_ctx = None
        kq_unit = None
        if only != "c":
            hq_ctx = tc.tile_pool(name="hqp", bufs=1)
            hq_pool = hq_ctx.__enter__()
            hq_sb = hq_pool.tile([128, MT, T], BF16, tag="hq")
            w_ctx = tc.tile_pool(name="wts", bufs=1)
            w_pool = w_ctx.__enter__()
            wk_sb = w_pool.tile([128, MT, MT, KT], BF16, tag="wk")
            wq_sb = w_pool.tile([128, MT, MT, KT], BF16, tag="wq")
            wv_ctx = tc.tile_pool(name="wvp", bufs=1)
            wv_pool = wv_ctx.__enter__()
            wv_sb = wv_pool.tile([128, MT, E], BF16, tag="wv")
            col_ctx = tc.tile_pool(name="col", bufs=2)
            col_pool = col_ctx.__enter__()
            vps_ctx = tc.tile_pool(name="vps", bufs=2, space="PSUM")
            vps = vps_ctx.__enter__()
            kqp_ctx = tc.tile_pool(name="kqp", bufs=2, space="PSUM")
            kqp = kqp_ctx.__enter__()

            for k in range(MT):
                nc.sync.dma_start(wv_sb[:, k, :], wvT[k * KT:(k + 1) * KT, :])
            nc.sync.dma_start(wk_sb[:], wkT[:])
            nc.sync.dma_start(wq_sb[:], wqT[:])

            def col_load(i):
                hc = col_pool.tile([128, MT, KT], BF16, tag="hc", name="hc")
                nc.sync.dma_start(hc[:], hidT[:, i])
                oc = col_pool.tile([128, MT, KT], BF16, tag="oc", name="oc")
                nc.sync.dma_start(oc[:], oqT[:, i])
                return hc, oc

            def kq_unit(m, which, pool, tag="kqu"):
                # which: 0 = kT ch0, 1 = kT ch1, 2 = qT
                w = wq_sb if which == 2 else wk_sb
                t0 = TL if which == 1 else 0
                ps = pool.tile([128, TL], F32, tag=tag, name="ps")
                for k in range(MT):
                    for c in range(2):  # ISA caps matmul moving dim at 512
                        nc.tensor.matmul(
                            ps[:, 512 * c:512 * (c + 1)], w[:, m, k, :],
                            hq_sb[:, k, t0 + 512 * c:t0 + 512 * (c + 1)],
                            start=(k == 0), stop=(k == MT - 1))
                if which == 2:
                    nc.vector.tensor_scalar(
                        qT_sb[:, m, :], ps[:], bq_t[:, m:m + 1],
                        None, _mb.AluOpType.add)
                else:
                    nc.vector.tensor_scalar(
                        kT_sb[:, m, t0:t0 + TL], ps[:], bk_t[:, m:m + 1],
                        None, _mb.AluOpType.add)

            # ch0 units for the pre-blocks (need hq cols 0..7): 1/i from i=10
            ch0_units = []
            for m in range(N_PRE):
                ch0_units.append((m, 0))
                ch0_units.append((m, 2))

            nxt = col_load(0)
            for i in range(ST):
                hc, oc = nxt
                if i + 1 < ST:
                    nxt = col_load(i + 1)
                nc.vector.tensor_add(hq_sb[:, :, i * KT:(i + 1) * KT],
                                     hc[:], oc[:])
                ps = vps.tile([128, E], F32, tag="vps", name="ps")
                for k in range(MT):
                    for c in range(2):
                        nc.tensor.matmul(
                            ps[:, 512 * c:512 * (c + 1)], hc[:, k, :],
                            wv_sb[:, k, 512 * c:512 * (c + 1)],
                            start=(k == 0), stop=(k == MT - 1))
                nc.vector.tensor_copy(
                    v_sb[:, i, :, 0:64],
                    ps[:].rearrange("p (h d) -> p h d", d=64))
                if i >= ST - len(ch0_units):
                    kq_unit(*ch0_units[i - (ST - len(ch0_units))], pool=kqp)
            for m in range(N_PRE):
                kq_unit(m, 1, kqp)
            if only == "ab":  # no attention to fill from: finish kq serially
                for m in range(N_PRE, MT):
                    for which in (0, 2, 1):
                        kq_unit(m, which, kqp)

            kqp_ctx.__exit__(None, None, None)
            vps_ctx.__exit__(None, None, None)
            col_ctx.__exit__(None, None, None)
            wv_ctx.__exit__(None, None, None)

        # =========== phase B: attention =====================================
        outT_ctx = tc.tile_pool(name="outT", bufs=1, side="right")
        outT_pool = outT_ctx.__enter__()
        outT_sb = outT_pool.tile([128, MT, TL], BF16, tag="outT")

        if only == "c":
            nc.vector.memset(kT_sb[:, 0, 0:16], 0.5)
            nc.vector.memset(qT_sb[:, 0, 0:16], 0.5)
            nc.vector.memset(v_sb[:, 0, 0, 0:16], 0.5)

        if only != "ab":
            exp_ctx = tc.tile_pool(name="expp", bufs=6)
            exp_pool = exp_ctx.__enter__()
            z_ctx = tc.tile_pool(name="zp", bufs=2)
            z_pool = z_ctx.__enter__()
            sc_ctx = tc.tile_pool(name="sc", bufs=2, space="PSUM")
            sc = sc_ctx.__enter__()
            pv_ctx = tc.tile_pool(name="pv", bufs=2, space="PSUM")
            pv = pv_ctx.__enter__()

            # kq fill units for blocks N_PRE..7, one per head boundary,
            # borrowing a pv-pool slot so no extra PSUM banks are needed.
            fills = []
            if only == "full":
                for m in range(N_PRE, MT):
                    for which in (0, 2, 1):
                        fills.append((m, which))
            fill_idx = 0
            pending = [None]  # epilogue of the previous head

            def epilogue(psv, lo, hi, p):
                def run():
                    rz = z_pool.tile([65, TL], F32R, tag="rz", name="rz")
                    with nc.allow_low_precision(reason="softmax recip"):
                        nc.vector.reciprocal(rz[64:65, :], psv[64:65, :])
                    pzb = pv.tile([128, TL], F32, tag="pv", name="pzb")
                    for c in range(2):
                        nc.tensor.matmul(pzb[0:64, 512 * c:512 * (c + 1)],
                                         ones65[64:65, :],
                                         rz[64:65, 512 * c:512 * (c + 1)],
                                         start=True, stop=True)
                    zbs = z_pool.tile([64, TL], F32, tag="zbs", name="zbs")
                    nc.vector.tensor_copy(zbs[:], pzb[0:64, :])
                    nc.vector.tensor_mul(
                        outT_sb[lo:hi, p, :], psv[0:64, :], zbs[:])
                return run

            for p in range(MT):
                for e in range(2):
                    h = 2 * p + e
                    lo, hi = 64 * e, 64 * e + 64
                    pst = pv.tile([128, TL], F32, tag="pv", name="psv")
                    psv = pst[0:65]
                    exs = [None] * ST

                    def att_mm(j, start, stop):
                        for c in range(2):
                            nc.tensor.matmul(
                                psv[:, 512 * c:512 * (c + 1)],
                                v_sb[:, j, h, :],
                                exs[j][:, 512 * c:512 * (c + 1)],
                                start=start, stop=stop)

                    for i in range(ST):
                        psc = sc.tile([128, TL], F32, tag="sc", name="psc")
                        for c in range(2):
                            nc.tensor.matmul(
                                psc[:, 512 * c:512 * (c + 1)],
                                kT_sb[lo:hi, p, i * KT:(i + 1) * KT],
                                qT_sb[lo:hi, p, 512 * c:512 * (c + 1)],
                                start=True, stop=True)
                        ex = exp_pool.tile([128, TL], BF16, tag="exp",
                                           name="ex")
                        nc.scalar.activation(ex[:], psc[:], EXP, scale=SCALE)
                        exs[i] = ex
                        if i == 1 and pending[0] is not None:
                            pending[0]()
                            pending[0] = None
                        if i == 2 and fill_idx < len(fills):
                            kq_unit(*fills[fill_idx], pool=pv, tag="pv")
                            fill_idx += 1
                        j = i - LAG
                        if j >= 0:
                            att_mm(j, j == 0, j == ST - 1)
                            exs[j] = None
                    for j in range(ST - LAG, ST):
                        att_mm(j, j == 0, j == ST - 1)
                    pending[0] = epilogue(psv, lo, hi, p)
            pending[0]()
            pending[0] = None

            pv_ctx.__exit__(None, None, None)
            sc_ctx.__exit__(None, None, None)
            z_ctx.__exit__(None, None, None)
            exp_ctx.__exit__(None, None, None)
        if w_ctx is not None:
            w_ctx.__exit__(None, None, None)
            hq_ctx.__exit__(None, None, None)

        # =========== phase C: out_proj ======================================
        if only == "ab":
            nc.vector.memset(outT_sb[:, 0, 0:16], 0.5)
        with tc.tile_pool(name="wo", bufs=1) as wo_pool, \
             tc.tile_pool(name="yo", bufs=3) as y_pool, \
             tc.tile_pool(name="yp", bufs=3, space="PSUM") as yp:
            wo_sb = wo_pool.tile([128, MT, E], BF16, tag="wo")
            for k in range(MT):
                nc.sync.dma_start(wo_sb[:, k, :], woT[k * KT:(k + 1) * KT, :])
            for tt in range(TL // KT):
                ps = yp.tile([128, E], F32, tag="yp", name="ps")
                for k in range(MT):
                    for c in range(2):
                        nc.tensor.matmul(
                            ps[:, 512 * c:512 * (c + 1)],
                            outT_sb[:, k, tt * KT:(tt + 1) * KT],
                            wo_sb[:, k, 512 * c:512 * (c + 1)],
                            start=(k == 0), stop=False)
                for c in range(2):
                    nc.tensor.matmul(
                        ps[:, 512 * c:512 * (c + 1)], ones_b[0:1, :],
                        bo2_t[0:1, 512 * c:512 * (c + 1)],
                        start=False, stop=True)
                yt = y_pool.tile([128, E], F32, tag="yt", name="yt")
                nc.vector.tensor_copy(yt[:], ps[:])
                nc.sync.dma_start(y[tt * KT:(tt + 1) * KT, :], yt[:])
        outT_ctx.__exit__(None, None, None)

    nc.compile()
    return nc


_NC_CACHE = None


def _get_program():
    global _NC_CACHE
    if _NC_CACHE is None:
        _NC_CACHE = build_program()
    return _NC_CACHE


def _bf16_np():
    import ml_dtypes
    return ml_dtypes.bfloat16


def make_in_maps(hidden_states, object_queries, Wq, bq, Wk, bk, Wv, bv, Wo, bo):
    """Host-side sharding/layout prep -> per-core input dicts."""
    bf = _bf16_np()
    bo2 = bo + bv @ Wo.T
    shared = {
        "wqT": np.ascontiguousarray(
            Wq.T.reshape(MT, 128, MT, KT).transpose(1, 2, 0, 3)).astype(bf),
        "wkT": np.ascontiguousarray(
            Wk.T.reshape(MT, 128, MT, KT).transpose(1, 2, 0, 3)).astype(bf),
        "wvT": np.ascontiguousarray(Wv.T).astype(bf),
        "woT": np.ascontiguousarray(Wo.T).astype(bf),
        "bq": np.ascontiguousarray(bq.reshape(MT, 128).T),
        "bk": np.ascontiguousarray(bk.reshape(MT, 128).T),
        "bo2": bo2[None, :].astype(bf),
        "onesf": np.ones((1, KT), np.float32),
        "onesb": np.ones((1, KT), bf),
        "onebf": np.ones((128, ST * H), bf),
    }
    in_maps = []
    for c in range(N_CORES):
        b, half = c // 2, c % 2
        toff = half * TL
        # rotate T so this core's query rows come first (attention over s is
        # permutation-invariant as long as k/v share the ordering)
        hid = np.concatenate([hidden_states[b, toff:], hidden_states[b, :toff]], 0)
        oq = np.concatenate([object_queries[b, toff:], object_queries[b, :toff]], 0)
        m = dict(shared)
        m["hidT"] = np.ascontiguousarray(
            hid.T.reshape(MT, 128, ST, KT).transpose(1, 2, 0, 3)).astype(bf)
        m["oqT"] = np.ascontiguousarray(
            oq.T.reshape(MT, 128, ST, KT).transpose(1, 2, 0, 3)).astype(bf)
        in_maps.append(m)
    return in_maps


def kernel(**inputs):
    nc = _get_program()
    in_maps = make_in_maps(**{k: np.asarray(v) for k, v in inputs.items()})
    res = run_bass_kernel_spmd(nc, in_maps, core_ids=list(range(N_CORES)))
    out = np.empty((B, T, E), np.float32)
    for c in range(N_CORES):
        b, half = c // 2, c % 2
        out[b, half * TL:(half + 1) * TL] = res.results[c]["y"]
    return out


# revision 18
# speedup vs baseline: 121.9115x; 1.1222x over previous
"""DETR self-attention (B=4, T=2048, E=1024, H=16) on 8 trn2 NeuronCores.

Sharding: core c handles batch c//2 and query-half c%2 (1024 query rows),
computing K/V for the full 2048-token sequence of its batch (duplicated
across the pair of cores — cheaper than an intra-pair collective).

v3 design (1024-wide PSUM tiles, software-pipelined epilogue, kq fill):
  phase A: stream hidT/oqT columns (bf16); hq = hid+oq; v-proj into
           [128,1024] PSUM tiles (2x512 matmuls share one stationary --
           the ISA caps the moving dim at 512); kT/qT for the first
           N_PRE e-blocks only, interleaved into the column stream.
           bv is folded into bo on the host (attn rows sum to 1 =>
           bv contributes exactly bv @ Wo.T to the output).
  phase B: per head: 16 scores matmul pairs (d=64 contraction) -> one
           1024-wide exp on ACT (scale=1/8 folded) -> lagged attn@v into
           a [65,1024] slice of a [128,1024] PSUM accumulator (row 64 =
           softmax Z via ones col). The 1/Z epilogue (DVE reciprocal,
           k=1 PE broadcast matmul, psv*(1/Z) -> outT bf16) is deferred
           into the next head's pipeline; the remaining kq e-blocks fill
           attention's PE slack one unit per head, borrowing pv-pool
           PSUM slots (ACT exp is the phase floor at ~1 us/[128,1024]).
  phase C: out_proj: y[t, e] = outT.T @ WoT + bo2, DMA out per t-tile.
exp(scores) never overflows: scores*0.125 ~ N(0, 0.82), |max| < 6.
"""
import os
import sys

if "/opt/trn_rl_repo" not in sys.path:
    sys.path.insert(0, "/opt/trn_rl_repo")

from contextlib import ExitStack, nullcontext

import numpy as np

import concourse.bass as bass
import concourse.tile as tile
from concourse import bacc, mybir
from concourse.bass_utils import run_bass_kernel_spmd

F32 = mybir.dt.float32
F32R = mybir.dt.float32r
BF16 = mybir.dt.bfloat16
EXP = mybir.ActivationFunctionType.Exp

B, T, E, H, D = 4, 2048, 1024, 16, 64
TL = T // 2          # local query rows per core
N_CORES = 8
KT = 128             # contraction tile
MT = E // KT         # 8 e-blocks
ST = T // KT         # 16 s-tiles
SCALE = 1.0 / 8.0    # D ** -0.5
LAG = int(os.environ.get('K_LAG', 2))


def build_program(repeat=1, only="full"):
    nc = bacc.Bacc("TRN2", target_bir_lowering=False, debug=False)

    import concourse.mybir as _mb

    hidT = nc.dram_tensor("hidT", [128, ST, MT, KT], BF16,
                          kind="ExternalInput").ap()
    oqT = nc.dram_tensor("oqT", [128, ST, MT, KT], BF16,
                         kind="ExternalInput").ap()
    wqT = nc.dram_tensor("wqT", [128, MT, MT, KT], BF16,
                         kind="ExternalInput").ap()
    wkT = nc.dram_tensor("wkT", [128, MT, MT, KT], BF16,
                         kind="ExternalInput").ap()
    wvT = nc.dram_tensor("wvT", [E, E], BF16, kind="ExternalInput").ap()
    woT = nc.dram_tensor("woT", [E, E], BF16, kind="ExternalInput").ap()
    bq = nc.dram_tensor("bq", [128, MT], F32, kind="ExternalInput").ap()
    bk = nc.dram_tensor("bk", [128, MT], F32, kind="ExternalInput").ap()
    bo2 = nc.dram_tensor("bo2", [1, E], BF16, kind="ExternalInput").ap()
    onesf = nc.dram_tensor("onesf", [1, KT], F32R, kind="ExternalInput").ap()
    onesb = nc.dram_tensor("onesb", [1, KT], BF16, kind="ExternalInput").ap()
    y = nc.dram_tensor("y", [TL, E], F32, kind="ExternalOutput").ap()

    with tile.TileContext(nc, pool_alloc_mode="queue") as tc, \
            (tc.For_i(0, repeat, 1) if repeat > 1 else nullcontext()), \
            ExitStack() as top:
        misc = top.enter_context(tc.tile_pool(name="misc", bufs=1))
        kq_pool = top.enter_context(tc.tile_pool(name="kq", bufs=1))
        v_pool = top.enter_context(tc.tile_pool(name="vp", bufs=1))

        # --- constants / biases ---
        ones65 = misc.tile([65, 64], F32R, tag="ones65")
        nc.sync.dma_start(ones65[64:65, :], onesf[0:1, 0:64])
        ones_b = misc.tile([1, KT], BF16, tag="onesb")
        nc.sync.dma_start(ones_b[:], onesb[:])
        bq_t = misc.tile([128, MT], F32, tag="bq")
        nc.sync.dma_start(bq_t[:], bq[:])
        bk_t = misc.tile([128, MT], F32, tag="bk")
        nc.sync.dma_start(bk_t[:], bk[:])
        bo2_t = misc.tile([1, E], BF16, tag="bo2")
        nc.sync.dma_start(bo2_t[:], bo2[:])

        # --- resident stores ---
        kT_sb = kq_pool.tile([128, MT, T], BF16, tag="kT")
        qT_sb = kq_pool.tile([128, MT, TL], BF16, tag="qT")
        v_sb = v_pool.tile([128, ST, H, 65], BF16, tag="v")
        nc.vector.memset(v_sb[:, :, :, 64:65], 1.0)

        # =========== phase A: v-proj + hq + kq ==============================
        N_PRE = 3  # kq blocks computed in phase A; the rest fill attention
        hq_ctx = w_ctx = None
        kq_unit = None
        if only != "c":
            hq_ctx = tc.tile_pool(name="hqp", bufs=1)
            hq_pool = hq_ctx.__enter__()
            hq_sb = hq_pool.tile([128, MT, T], BF16, tag="hq")
            w_ctx = tc.tile_pool(name="wts", bufs=1)
            w_pool = w_ctx.__enter__()
            wk_sb = w_pool.tile([128, MT, MT, KT], BF16, tag="wk")
            wq_sb = w_pool.tile([128, MT, MT, KT], BF16, tag="wq")
            wv_ctx = tc.tile_pool(name="wvp", bufs=1)
            wv_pool = wv_ctx.__enter__()
            wv_sb = wv_pool.tile([128, MT, E], BF16, tag="wv")
            col_ctx = tc.tile_pool(name="col", bufs=2)
            col_pool = col_ctx.__enter__()
            vps_ctx = tc.tile_pool(name="vps", bufs=2, space="PSUM")
            vps = vps_ctx.__enter__()
            kqp_ctx = tc.tile_pool(name="kqp", bufs=2, space="PSUM")
            kqp = kqp_ctx.__enter__()

            def col_load(i):
                hc = col_pool.tile([128, MT, KT], BF16, tag="hc", name="hc")
                nc.sync.dma_start(hc[:], hidT[:, i])
                oc = col_pool.tile([128, MT, KT], BF16, tag="oc", name="oc")
                nc.sync.dma_start(oc[:], oqT[:, i])
                return hc, oc

            def kq_unit(m, which, pool, tag="kqu"):
                # which: 0 = kT ch0, 1 = kT ch1, 2 = qT
                w = wq_sb if which == 2 else wk_sb
                t0 = TL if which == 1 else 0
                ps = pool.tile([128, TL], F32, tag=tag, name="ps")
                for k in range(MT):
                    for c in range(2):  # ISA caps matmul moving dim at 512
                        nc.tensor.matmul(
                            ps[:, 512 * c:512 * (c + 1)], w[:, m, k, :],
                            hq_sb[:, k, t0 + 512 * c:t0 + 512 * (c + 1)],
                            start=(k == 0), stop=(k == MT - 1))
                if which == 2:
                    nc.vector.tensor_scalar(
                        qT_sb[:, m, :], ps[:], bq_t[:, m:m + 1],
                        None, _mb.AluOpType.add)
                else:
                    nc.vector.tensor_scalar(
                        kT_sb[:, m, t0:t0 + TL], ps[:], bk_t[:, m:m + 1],
                        None, _mb.AluOpType.add)

            # ch0 units for the pre-blocks (need hq cols 0..7): 1/i from i=10
            ch0_units = []
            for m in range(N_PRE):
                ch0_units.append((m, 0))
                ch0_units.append((m, 2))

            nxt = col_load(0)
            nxt2 = col_load(1)
            for k in range(MT):
                nc.sync.dma_start(wv_sb[:, k, :], wvT[k * KT:(k + 1) * KT, :])
            for i in range(ST):
                hc, oc = nxt
                nxt = nxt2
                if i + 2 < ST:
                    nxt2 = col_load(i + 2)
                if i < MT:  # stream kq weights behind the columns
                    nc.sync.dma_start(wk_sb[:, i], wkT[:, i])
                    nc.sync.dma_start(wq_sb[:, i], wqT[:, i])
                nc.vector.tensor_add(hq_sb[:, :, i * KT:(i + 1) * KT],
                                     hc[:], oc[:])
                ps = vps.tile([128, E], F32, tag="vps", name="ps")
                for k in range(MT):
                    for c in range(2):
                        nc.tensor.matmul(
                            ps[:, 512 * c:512 * (c + 1)], hc[:, k, :],
                            wv_sb[:, k, 512 * c:512 * (c + 1)],
                            start=(k == 0), stop=(k == MT - 1))
                nc.vector.tensor_copy(
                    v_sb[:, i, :, 0:64],
                    ps[:].rearrange("p (h d) -> p h d", d=64))
                if i >= ST - len(ch0_units):
                    kq_unit(*ch0_units[i - (ST - len(ch0_units))], pool=kqp)
            for m in range(N_PRE):
                kq_unit(m, 1, kqp)
            if only == "ab":  # no attention to fill from: finish kq serially
                for m in range(N_PRE, MT):
                    for which in (0, 2, 1):
                        kq_unit(m, which, kqp)

            kqp_ctx.__exit__(None, None, None)
            vps_ctx.__exit__(None, None, None)
            col_ctx.__exit__(None, None, None)
            wv_ctx.__exit__(None, None, None)

        # =========== phase B: attention =====================================
        pending = [None]  # epilogue of the last head, flushed in phase C
        outT_ctx = tc.tile_pool(name="outT", bufs=1, side="right")
        outT_pool = outT_ctx.__enter__()
        outT_sb = outT_pool.tile([128, MT, TL], BF16, tag="outT")
        wo_ctx = tc.tile_pool(name="wo", bufs=1, side="right")
        wo_pool = wo_ctx.__enter__()
        wo_sb = wo_pool.tile([128, MT, E], BF16, tag="wo")
        for k in range(MT):
            nc.sync.dma_start(wo_sb[:, k, :], woT[k * KT:(k + 1) * KT, :])

        if only == "c":
            nc.vector.memset(kT_sb[:, 0, 0:16], 0.5)
            nc.vector.memset(qT_sb[:, 0, 0:16], 0.5)
            nc.vector.memset(v_sb[:, 0, 0, 0:16], 0.5)

        if only != "ab":
            z_ctx = tc.tile_pool(name="zp", bufs=1)
            z_pool = z_ctx.__enter__()
            pv_ctx = tc.tile_pool(name="pv", bufs=2, space="PSUM")
            pv = pv_ctx.__enter__()
            exp_ctx = tc.tile_pool(name="expp", bufs=6)
            exp_pool = exp_ctx.__enter__()
            sc_ctx = tc.tile_pool(name="sc", bufs=2, space="PSUM")
            sc = sc_ctx.__enter__()

            # kq fill units for blocks N_PRE..7, one per head boundary,
            # borrowing a pv-pool slot so no extra PSUM banks are needed.
            fills = []
            if only == "full":
                for m in range(N_PRE, MT):
                    for which in (0, 2, 1):
                        fills.append((m, which))
            fill_idx = 0

            def epilogue(psv, lo, hi, p):
                def run():
                    rz = z_pool.tile([65, TL], F32R, tag="rz", name="rz")
                    with nc.allow_low_precision(reason="softmax recip"):
                        nc.vector.reciprocal(rz[64:65, :], psv[64:65, :])
                    pzb = pv.tile([128, TL], F32, tag="pv", name="pzb")
                    for c in range(2):
                        nc.tensor.matmul(pzb[0:64, 512 * c:512 * (c + 1)],
                                         ones65[64:65, :],
                                         rz[64:65, 512 * c:512 * (c + 1)],
                                         start=True, stop=True)
                    zbs = z_pool.tile([64, TL], F32, tag="zbs", name="zbs")
                    nc.vector.tensor_copy(zbs[:], pzb[0:64, :])
                    nc.vector.tensor_mul(
                        outT_sb[lo:hi, p, :], psv[0:64, :], zbs[:])
                return run

            for p in range(MT):
                for e in range(2):
                    h = 2 * p + e
                    lo, hi = 64 * e, 64 * e + 64
                    pst = pv.tile([128, TL], F32, tag="pv", name="psv")
                    psv = pst[0:65]
                    exs = [None] * ST

                    def att_mm(j, start, stop):
                        for c in range(2):
                            nc.tensor.matmul(
                                psv[:, 512 * c:512 * (c + 1)],
                                v_sb[:, j, h, :],
                                exs[j][:, 512 * c:512 * (c + 1)],
                                start=start, stop=stop)

                    for i in range(ST):
                        psc = sc.tile([128, TL], F32, tag="sc", name="psc")
                        for c in range(2):
                            nc.tensor.matmul(
                                psc[:, 512 * c:512 * (c + 1)],
                                kT_sb[lo:hi, p, i * KT:(i + 1) * KT],
                                qT_sb[lo:hi, p, 512 * c:512 * (c + 1)],
                                start=True, stop=True)
                        ex = exp_pool.tile([128, TL], BF16, tag="exp",
                                           name="ex")
                        nc.scalar.activation(ex[:], psc[:], EXP, scale=SCALE)
                        exs[i] = ex
                        if i == 1 and pending[0] is not None:
                            pending[0]()
                            pending[0] = None
                        if i == 2 and fill_idx < len(fills):
                            kq_unit(*fills[fill_idx], pool=pv, tag="pv")
                            fill_idx += 1
                        j = i - LAG
                        if j >= 0:
                            att_mm(j, j == 0, j == ST - 1)
                            exs[j] = None
                    for j in range(ST - LAG, ST):
                        att_mm(j, j == 0, j == ST - 1)
                    pending[0] = epilogue(psv, lo, hi, p)

            sc_ctx.__exit__(None, None, None)
            exp_ctx.__exit__(None, None, None)

        # =========== phase C: out_proj ======================================
        if only == "ab":
            nc.vector.memset(outT_sb[:, 0, 0:16], 0.5)
        with tc.tile_pool(name="yo", bufs=3) as y_pool, \
             tc.tile_pool(name="yp", bufs=2, space="PSUM") as yp:
            for tt in range(TL // KT):
                ps = yp.tile([128, E], F32, tag="yp", name="ps")
                for k in range(MT):
                    if tt == 0 and k == 6 and pending[0] is not None:
                        pending[0]()  # head-15 epilogue hides under k=0..5
                        pending[0] = None
                    for c in range(2):
                        nc.tensor.matmul(
                            ps[:, 512 * c:512 * (c + 1)],
                            outT_sb[:, k, tt * KT:(tt + 1) * KT],
                            wo_sb[:, k, 512 * c:512 * (c + 1)],
                            start=(k == 0), stop=False)
                for c in range(2):
                    nc.tensor.matmul(
                        ps[:, 512 * c:512 * (c + 1)], ones_b[0:1, :],
                        bo2_t[0:1, 512 * c:512 * (c + 1)],
                        start=False, stop=True)
                yt = y_pool.tile([128, E], F32, tag="yt", name="yt")
                nc.vector.tensor_copy(yt[:], ps[:])
                nc.sync.dma_start(y[tt * KT:(tt + 1) * KT, :], yt[:])
        if only != "ab":
            pv_ctx.__exit__(None, None, None)
            z_ctx.__exit__(None, None, None)
        if w_ctx is not None:
            w_ctx.__exit__(None, None, None)
            hq_ctx.__exit__(None, None, None)
        wo_ctx.__exit__(None, None, None)
        outT_ctx.__exit__(None, None, None)

    nc.compile()
    return nc


_NC_CACHE = None


def _get_program():
    global _NC_CACHE
    if _NC_CACHE is None:
        _NC_CACHE = build_program()
    return _NC_CACHE


def _bf16_np():
    import ml_dtypes
    return ml_dtypes.bfloat16


def make_in_maps(hidden_states, object_queries, Wq, bq, Wk, bk, Wv, bv, Wo, bo):
    """Host-side sharding/layout prep -> per-core input dicts."""
    bf = _bf16_np()
    bo2 = bo + bv @ Wo.T
    shared = {
        "wqT": np.ascontiguousarray(
            Wq.T.reshape(MT, 128, MT, KT).transpose(1, 2, 0, 3)).astype(bf),
        "wkT": np.ascontiguousarray(
            Wk.T.reshape(MT, 128, MT, KT).transpose(1, 2, 0, 3)).astype(bf),
        "wvT": np.ascontiguousarray(Wv.T).astype(bf),
        "woT": np.ascontiguousarray(Wo.T).astype(bf),
        "bq": np.ascontiguousarray(bq.reshape(MT, 128).T),
        "bk": np.ascontiguousarray(bk.reshape(MT, 128).T),
        "bo2": bo2[None, :].astype(bf),
        "onesf": np.ones((1, KT), np.float32),
        "onesb": np.ones((1, KT), bf),
    }
    in_maps = []
    for c in range(N_CORES):
        b, half = c // 2, c % 2
        toff = half * TL
        # rotate T so this core's query rows come first (attention over s is
        # permutation-invariant as long as k/v share the ordering)
        hid = np.concatenate([hidden_states[b, toff:], hidden_states[b, :toff]], 0)
        oq = np.concatenate([object_queries[b, toff:], object_queries[b, :toff]], 0)
        m = dict(shared)
        m["hidT"] = np.ascontiguousarray(
            hid.T.reshape(MT, 128, ST, KT).transpose(1, 2, 0, 3)).astype(bf)
        m["oqT"] = np.ascontiguousarray(
            oq.T.reshape(MT, 128, ST, KT).transpose(1, 2, 0, 3)).astype(bf)
        in_maps.append(m)
    return in_maps


def kernel(**inputs):
    nc = _get_program()
    in_maps = make_in_maps(**{k: np.asarray(v) for k, v in inputs.items()})
    res = run_bass_kernel_spmd(nc, in_maps, core_ids=list(range(N_CORES)))
    out = np.empty((B, T, E), np.float32)
    for c in range(N_CORES):
        b, half = c // 2, c % 2
        out[b, half * TL:(half + 1) * TL] = res.results[c]["y"]
    return out


# revision 24
# speedup vs baseline: 125.5111x; 1.0295x over previous
"""DETR self-attention (B=4, T=2048, E=1024, H=16) on 8 trn2 NeuronCores.

Sharding: core c handles batch c//2 and query-half c%2 (1024 query rows),
computing K/V for the full 2048-token sequence of its batch (duplicated
across the pair of cores — cheaper than an intra-pair collective).

v3 design (1024-wide PSUM tiles, software-pipelined epilogue, kq fill):
  phase A: stream hidT/oqT columns (bf16); hq = hid+oq; v-proj into
           [128,1024] PSUM tiles (2x512 matmuls share one stationary --
           the ISA caps the moving dim at 512); kT/qT for the first
           N_PRE e-blocks only, interleaved into the column stream.
           bv is folded into bo on the host (attn rows sum to 1 =>
           bv contributes exactly bv @ Wo.T to the output).
  phase B: per head: 16 scores matmul pairs (d=64 contraction) -> one
           1024-wide exp on ACT (scale=1/8 folded) -> lagged attn@v into
           a [65,1024] slice of a [128,1024] PSUM accumulator (row 64 =
           softmax Z via ones col). The 1/Z epilogue (DVE reciprocal,
           k=1 PE broadcast matmul, psv*(1/Z) -> outT bf16) is deferred
           into the next head's pipeline; the remaining kq e-blocks fill
           attention's PE slack one unit per head, borrowing pv-pool
           PSUM slots (ACT exp is the phase floor at ~1 us/[128,1024]).
  phase C: out_proj: y[t, e] = outT.T @ WoT + bo2, DMA out per t-tile.
exp(scores) never overflows: scores*0.125 ~ N(0, 0.82), |max| < 6.
"""
import os
import sys

if "/opt/trn_rl_repo" not in sys.path:
    sys.path.insert(0, "/opt/trn_rl_repo")

from contextlib import ExitStack, nullcontext

import numpy as np

import concourse.bass as bass
import concourse.tile as tile
from concourse import bacc, mybir
from concourse.bass_utils import run_bass_kernel_spmd

F32 = mybir.dt.float32
F32R = mybir.dt.float32r
BF16 = mybir.dt.bfloat16
EXP = mybir.ActivationFunctionType.Exp

B, T, E, H, D = 4, 2048, 1024, 16, 64
TL = T // 2          # local query rows per core
N_CORES = 8
KT = 128             # contraction tile
MT = E // KT         # 8 e-blocks
ST = T // KT         # 16 s-tiles
SCALE = 1.0 / 8.0    # D ** -0.5
LAG = int(os.environ.get('K_LAG', 2))


def build_program(repeat=1, only="full"):
    nc = bacc.Bacc("TRN2", target_bir_lowering=False, debug=False)

    import concourse.mybir as _mb

    hidT = nc.dram_tensor("hidT", [128, ST, MT, KT], BF16,
                          kind="ExternalInput").ap()
    oqT = nc.dram_tensor("oqT", [128, ST, MT, KT], BF16,
                         kind="ExternalInput").ap()
    wqT = nc.dram_tensor("wqT", [128, MT, MT, KT], BF16,
                         kind="ExternalInput").ap()
    wkT = nc.dram_tensor("wkT", [128, MT, MT, KT], BF16,
                         kind="ExternalInput").ap()
    wvT = nc.dram_tensor("wvT", [E, E], BF16, kind="ExternalInput").ap()
    woT = nc.dram_tensor("woT", [E, E], BF16, kind="ExternalInput").ap()
    bq = nc.dram_tensor("bq", [128, MT], F32, kind="ExternalInput").ap()
    bk = nc.dram_tensor("bk", [128, MT], F32, kind="ExternalInput").ap()
    bo2 = nc.dram_tensor("bo2", [1, E], BF16, kind="ExternalInput").ap()
    onesf = nc.dram_tensor("onesf", [1, KT], F32R, kind="ExternalInput").ap()
    onesb = nc.dram_tensor("onesb", [1, KT], BF16, kind="ExternalInput").ap()
    y = nc.dram_tensor("y", [TL, E], F32, kind="ExternalOutput").ap()

    with tile.TileContext(nc, pool_alloc_mode="queue") as tc, \
            (tc.For_i(0, repeat, 1) if repeat > 1 else nullcontext()), \
            ExitStack() as top:
        misc = top.enter_context(tc.tile_pool(name="misc", bufs=1))
        kq_pool = top.enter_context(tc.tile_pool(name="kq", bufs=1))
        v_pool = top.enter_context(tc.tile_pool(name="vp", bufs=1))

        # --- constants / biases ---
        ones_b = misc.tile([1, KT], BF16, tag="onesb")
        nc.sync.dma_start(ones_b[:], onesb[:])
        bq_t = misc.tile([128, MT], F32, tag="bq")
        nc.sync.dma_start(bq_t[:], bq[:])
        bk_t = misc.tile([128, MT], F32, tag="bk")
        nc.sync.dma_start(bk_t[:], bk[:])
        bo2_t = misc.tile([1, E], BF16, tag="bo2")
        nc.sync.dma_start(bo2_t[:], bo2[:])

        # --- resident stores ---
        kT_sb = kq_pool.tile([128, MT, T], BF16, tag="kT")
        qT_sb = kq_pool.tile([128, MT, TL], BF16, tag="qT")
        v_sb = v_pool.tile([128, ST, H, 65], BF16, tag="v")
        nc.vector.memset(v_sb[:, :, :, 64:65], 1.0)

        # =========== phase A: v-proj + hq + kq ==============================
        N_PRE = 3  # kq blocks computed in phase A; the rest fill attention
        hq_ctx = w_ctx = None
        kq_unit = None
        if only != "c":
            hq_ctx = tc.tile_pool(name="hqp", bufs=1)
            hq_pool = hq_ctx.__enter__()
            hq_sb = hq_pool.tile([128, MT, T], BF16, tag="hq")
            w_ctx = tc.tile_pool(name="wts", bufs=1)
            w_pool = w_ctx.__enter__()
            wk_sb = w_pool.tile([128, MT, MT, KT], BF16, tag="wk")
            wq_sb = w_pool.tile([128, MT, MT, KT], BF16, tag="wq")
            wv_ctx = tc.tile_pool(name="wvp", bufs=1)
            wv_pool = wv_ctx.__enter__()
            wv_sb = wv_pool.tile([128, MT, E], BF16, tag="wv")
            col_ctx = tc.tile_pool(name="col", bufs=2)
            col_pool = col_ctx.__enter__()
            vps_ctx = tc.tile_pool(name="vps", bufs=2, space="PSUM")
            vps = vps_ctx.__enter__()
            kqp_ctx = tc.tile_pool(name="kqp", bufs=2, space="PSUM")
            kqp = kqp_ctx.__enter__()

            def col_load(i):
                hc = col_pool.tile([128, MT, KT], BF16, tag="hc", name="hc")
                nc.sync.dma_start(hc[:], hidT[:, i])
                oc = col_pool.tile([128, MT, KT], BF16, tag="oc", name="oc")
                nc.sync.dma_start(oc[:], oqT[:, i])
                return hc, oc

            def kq_unit(m, which, pool, tag="kqu"):
                # which: 0 = kT ch0, 1 = kT ch1, 2 = qT
                w = wq_sb if which == 2 else wk_sb
                t0 = TL if which == 1 else 0
                ps = pool.tile([128, TL], F32, tag=tag, name="ps")
                for k in range(MT):
                    for c in range(2):  # ISA caps matmul moving dim at 512
                        nc.tensor.matmul(
                            ps[:, 512 * c:512 * (c + 1)], w[:, m, k, :],
                            hq_sb[:, k, t0 + 512 * c:t0 + 512 * (c + 1)],
                            start=(k == 0), stop=(k == MT - 1))
                if which == 2:
                    nc.vector.tensor_scalar(
                        qT_sb[:, m, :], ps[:], bq_t[:, m:m + 1],
                        None, _mb.AluOpType.add)
                else:
                    nc.vector.tensor_scalar(
                        kT_sb[:, m, t0:t0 + TL], ps[:], bk_t[:, m:m + 1],
                        None, _mb.AluOpType.add)

            # ch0 units for the pre-blocks (need hq cols 0..7): 1/i from i=10
            ch0_units = []
            for m in range(N_PRE):
                ch0_units.append((m, 0))
                ch0_units.append((m, 2))

            nxt = col_load(0)
            nxt2 = col_load(1)
            for k in range(MT):
                nc.sync.dma_start(wv_sb[:, k, :], wvT[k * KT:(k + 1) * KT, :])
            for i in range(ST):
                hc, oc = nxt
                nxt = nxt2
                if i + 2 < ST:
                    nxt2 = col_load(i + 2)
                if i < MT:  # stream kq weights behind the columns
                    nc.sync.dma_start(wk_sb[:, i], wkT[:, i])
                    nc.sync.dma_start(wq_sb[:, i], wqT[:, i])
                nc.vector.tensor_add(hq_sb[:, :, i * KT:(i + 1) * KT],
                                     hc[:], oc[:])
                ps = vps.tile([128, E], F32, tag="vps", name="ps")
                for k in range(MT):
                    for c in range(2):
                        nc.tensor.matmul(
                            ps[:, 512 * c:512 * (c + 1)], hc[:, k, :],
                            wv_sb[:, k, 512 * c:512 * (c + 1)],
                            start=(k == 0), stop=(k == MT - 1))
                nc.vector.tensor_copy(
                    v_sb[:, i, :, 0:64],
                    ps[:].rearrange("p (h d) -> p h d", d=64))
                if i >= ST - len(ch0_units):
                    kq_unit(*ch0_units[i - (ST - len(ch0_units))], pool=kqp)
            for m in range(N_PRE):
                kq_unit(m, 1, kqp)
            if only == "ab":  # no attention to fill from: finish kq serially
                for m in range(N_PRE, MT):
                    for which in (0, 2, 1):
                        kq_unit(m, which, kqp)

            kqp_ctx.__exit__(None, None, None)
            vps_ctx.__exit__(None, None, None)
            col_ctx.__exit__(None, None, None)
            wv_ctx.__exit__(None, None, None)

        # =========== phase B: attention =====================================
        pending = [None]  # epilogue of the last head, flushed in phase C
        outT_ctx = tc.tile_pool(name="outT", bufs=1, side="right")
        outT_pool = outT_ctx.__enter__()
        outT_sb = outT_pool.tile([128, MT, TL], BF16, tag="outT")
        wo_ctx = tc.tile_pool(name="wo", bufs=1, side="right")
        wo_pool = wo_ctx.__enter__()
        wo_sb = wo_pool.tile([128, MT, E], BF16, tag="wo")
        for k in range(MT):
            nc.sync.dma_start(wo_sb[:, k, :], woT[k * KT:(k + 1) * KT, :])

        if only == "c":
            nc.vector.memset(kT_sb[:, 0, 0:16], 0.5)
            nc.vector.memset(qT_sb[:, 0, 0:16], 0.5)
            nc.vector.memset(v_sb[:, 0, 0, 0:16], 0.5)

        if only != "ab":
            z_ctx = tc.tile_pool(name="zp", bufs=1)
            z_pool = z_ctx.__enter__()
            pv_ctx = tc.tile_pool(name="pv", bufs=2, space="PSUM")
            pv = pv_ctx.__enter__()
            exp_ctx = tc.tile_pool(name="expp", bufs=6)
            exp_pool = exp_ctx.__enter__()
            sc_ctx = tc.tile_pool(name="sc", bufs=2, space="PSUM")
            sc = sc_ctx.__enter__()

            # kq fill units for blocks N_PRE..7, one per head boundary,
            # borrowing a pv-pool slot so no extra PSUM banks are needed.
            fills = []
            if only == "full":
                for m in range(N_PRE, MT):
                    for which in (0, 2, 1):
                        fills.append((m, which))
            fill_idx = 0

            def epilogue(psv, lo, hi, p):
                def run():
                    rz = z_pool.tile([1, TL], F32, tag="rz", name="rz")
                    with nc.allow_low_precision(reason="softmax recip"):
                        nc.vector.reciprocal(rz[0:1, :], psv[64:65, :])
                    zbs = z_pool.tile([64, TL], F32, tag="zbs", name="zbs")
                    nc.gpsimd.partition_broadcast(zbs[:], rz[0:1, :])
                    nc.vector.tensor_mul(
                        outT_sb[lo:hi, p, :], psv[0:64, :], zbs[:])
                return run

            for p in range(MT):
                for e in range(2):
                    h = 2 * p + e
                    lo, hi = 64 * e, 64 * e + 64
                    pst = pv.tile([128, TL], F32, tag="pv", name="psv")
                    psv = pst[0:65]
                    exs = [None] * ST

                    def att_mm(j, start, stop):
                        for c in range(2):
                            nc.tensor.matmul(
                                psv[:, 512 * c:512 * (c + 1)],
                                v_sb[:, j, h, :],
                                exs[j][:, 512 * c:512 * (c + 1)],
                                start=start, stop=stop)

                    for i in range(ST):
                        psc = sc.tile([128, TL], F32, tag="sc", name="psc")
                        for c in range(2):
                            nc.tensor.matmul(
                                psc[:, 512 * c:512 * (c + 1)],
                                kT_sb[lo:hi, p, i * KT:(i + 1) * KT],
                                qT_sb[lo:hi, p, 512 * c:512 * (c + 1)],
                                start=True, stop=True)
                        ex = exp_pool.tile([128, TL], BF16, tag="exp",
                                           name="ex")
                        nc.scalar.activation(ex[:], psc[:], EXP, scale=SCALE)
                        exs[i] = ex
                        if i == 1 and pending[0] is not None:
                            pending[0]()
                            pending[0] = None
                        if i == 2 and fill_idx < len(fills):
                            kq_unit(*fills[fill_idx], pool=pv, tag="pv")
                            fill_idx += 1
                        j = i - LAG
                        if j >= 0:
                            att_mm(j, j == 0, j == ST - 1)
                            exs[j] = None
                    for j in range(ST - LAG, ST):
                        att_mm(j, j == 0, j == ST - 1)
                    pending[0] = epilogue(psv, lo, hi, p)

            sc_ctx.__exit__(None, None, None)
            exp_ctx.__exit__(None, None, None)

        # =========== phase C: out_proj ======================================
        if only == "ab":
            nc.vector.memset(outT_sb[:, 0, 0:16], 0.5)
        with tc.tile_pool(name="yo", bufs=3) as y_pool, \
             tc.tile_pool(name="yp", bufs=2, space="PSUM") as yp:
            for tt in range(TL // KT):
                ps = yp.tile([128, E], F32, tag="yp", name="ps")
                for k in range(MT):
                    if tt == 0 and k == 6 and pending[0] is not None:
                        pending[0]()  # head-15 epilogue hides under k=0..5
                        pending[0] = None
                    for c in range(2):
                        nc.tensor.matmul(
                            ps[:, 512 * c:512 * (c + 1)],
                            outT_sb[:, k, tt * KT:(tt + 1) * KT],
                            wo_sb[:, k, 512 * c:512 * (c + 1)],
                            start=(k == 0), stop=False)
                for c in range(2):
                    nc.tensor.matmul(
                        ps[:, 512 * c:512 * (c + 1)], ones_b[0:1, :],
                        bo2_t[0:1, 512 * c:512 * (c + 1)],
                        start=False, stop=True)
                yt = y_pool.tile([128, E], F32, tag="yt", name="yt")
                nc.vector.tensor_copy(yt[:], ps[:])
                nc.sync.dma_start(y[tt * KT:(tt + 1) * KT, :], yt[:])
        if only != "ab":
            pv_ctx.__exit__(None, None, None)
            z_ctx.__exit__(None, None, None)
        if w_ctx is not None:
            w_ctx.__exit__(None, None, None)
            hq_ctx.__exit__(None, None, None)
        wo_ctx.__exit__(None, None, None)
        outT_ctx.__exit__(None, None, None)

    nc.compile()
    return nc


_NC_CACHE = None


def _get_program():
    global _NC_CACHE
    if _NC_CACHE is None:
        _NC_CACHE = build_program()
    return _NC_CACHE


def _bf16_np():
    import ml_dtypes
    return ml_dtypes.bfloat16


def make_in_maps(hidden_states, object_queries, Wq, bq, Wk, bk, Wv, bv, Wo, bo):
    """Host-side sharding/layout prep -> per-core input dicts."""
    bf = _bf16_np()
    bo2 = bo + bv @ Wo.T
    shared = {
        "wqT": np.ascontiguousarray(
            Wq.T.reshape(MT, 128, MT, KT).transpose(1, 2, 0, 3)).astype(bf),
        "wkT": np.ascontiguousarray(
            Wk.T.reshape(MT, 128, MT, KT).transpose(1, 2, 0, 3)).astype(bf),
        "wvT": np.ascontiguousarray(Wv.T).astype(bf),
        "woT": np.ascontiguousarray(Wo.T).astype(bf),
        "bq": np.ascontiguousarray(bq.reshape(MT, 128).T),
        "bk": np.ascontiguousarray(bk.reshape(MT, 128).T),
        "bo2": bo2[None, :].astype(bf),
        "onesf": np.ones((1, KT), np.float32),
        "onesb": np.ones((1, KT), bf),
    }
    in_maps = []
    for c in range(N_CORES):
        b, half = c // 2, c % 2
        toff = half * TL
        # rotate T so this core's query rows come first (attention over s is
        # permutation-invariant as long as k/v share the ordering)
        hid = np.concatenate([hidden_states[b, toff:], hidden_states[b, :toff]], 0)
        oq = np.concatenate([object_queries[b, toff:], object_queries[b, :toff]], 0)
        m = dict(shared)
        m["hidT"] = np.ascontiguousarray(
            hid.T.reshape(MT, 128, ST, KT).transpose(1, 2, 0, 3)).astype(bf)
        m["oqT"] = np.ascontiguousarray(
            oq.T.reshape(MT, 128, ST, KT).transpose(1, 2, 0, 3)).astype(bf)
        in_maps.append(m)
    return in_maps


def kernel(**inputs):
    nc = _get_program()
    in_maps = make_in_maps(**{k: np.asarray(v) for k, v in inputs.items()})
    res = run_bass_kernel_spmd(nc, in_maps, core_ids=list(range(N_CORES)))
    out = np.empty((B, T, E), np.float32)
    for c in range(N_CORES):
        b, half = c // 2, c % 2
        out[b, half * TL:(half + 1) * TL] = res.results[c]["y"]
    return out
